# revision 4
# baseline (speedup 1.0000x reference)
"""FFM (field-aware factorization machine) forward kernel for 8 TRN2 NeuronCores.

y[b] = x[b] @ w_lin + b_lin + sum_{i<j} Wu[i,j] x[b,i] x[b,j]
with Wu = triu(Wmat, 1), Wmat[i,j] = <v[i, field[j]], v[j, field[i]]>.

Strategy (v5):
  - Host: build Wmat from (v, field_idx) [tiny], symmetrize
    S = (Wu + Wu^T)/2, eigendecompose S = Q diag(lam) Q^T. Shift the
    spectrum by c = -lam_min so mu = lam + c >= 0, fold sqrt(mu) into the
    eigenvectors: Q' = Q diag(sqrt(mu)). Then
      x^T Wu x = sum_n mu_n (x . q_n)^2 - c ||x||^2
    and the -c||x||^2 correction joins the (host-computed) linear part.
  - Device (data-parallel over batch, 8 cores): per 128-sample chunk,
    PE computes z = x_chunk^T Q' with batch on PSUM partitions and the
    eigen index on the free dim (two bf16 matmuls, contraction 256).
    The per-sample reduction sum_n z_n^2 runs along the FREE dim split
    across two engines:
      * ACT chunks: ScalarE Square(psum)+accum_out (1 instr/chunk,
        ~(256+111)/1.2 + 32 + 187(acc-read) ns).
      * DVE chunks: VectorE bn_stats over PAIRS of chunks (FD=512 is
        the HW max) reading z straight from PSUM; Sum z^2 is then
        reconstructed from the even/odd (count,mean,count*var) stats in
        one batched 5-op fixup at rep end:
          sum z^2 = cv_e + 128 me^2 + cv_o + 128 mo^2.
        This avoids the PSUM->SBUF copy entirely (PSUM may be read once
        per DVE instr, but bn_stats needs it only once).
  - x ships as bf16 pre-transposed (contraction on partitions) in
    single-DMA-per-slab layout; slabs alternate between the two HWDGE
    rings (SP + ACT sequencers) to raise effective HBM bandwidth.
  - y columns are engine-ordered (ACT cols first, then DVE cols);
    the host inverts the permutation.
"""

import numpy as np

_B, _N = 65536, 256
_NCORES = 8
_BS = _B // _NCORES   # 8192 samples per core
_NCH = _BS // 128     # 64 batch chunks per core

_compiled_nc = {}


def _assign(act_chunks, gsz, nch=_NCH):
    """Distribute ACT-reduced chunks evenly across groups.

    Returns (n_act_of[g], act_order, dve_order) where act_order/dve_order
    list global chunk ids in the order their y values appear in
    y_act cols / stats slots."""
    n_groups = nch // gsz
    base, rem = divmod(act_chunks, n_groups)
    n_act_of = [base + (1 if (g * rem) % n_groups < rem else 0)
                for g in range(n_groups)]
    # fix rounding to hit the exact total
    total = sum(n_act_of)
    g = 0
    while total > act_chunks:
        if n_act_of[g] > 0:
            n_act_of[g] -= 1
            total -= 1
        g = (g + 1) % n_groups
    while total < act_chunks:
        if n_act_of[g] < gsz:
            n_act_of[g] += 1
            total += 1
        g = (g + 1) % n_groups
    act_order, dve_order = [], []
    for g in range(n_groups):
        for j in range(gsz):
            c = g * gsz + j
            if j < n_act_of[g]:
                act_order.append(c)
            else:
                dve_order.append(c)
    return n_act_of, act_order, dve_order


# DMA column schedule: uniform slabs keep the DMA transfer stream matched
# to PE consumption. Must sum to _BS.
_DMA_SCHED = (1024,) * 8


def _build_nc(reps=1, mode="full", act_chunks=28, xin_bufs=3,
              gsz=4, pz_bufs=2, sched=_DMA_SCHED, dma_alt=True):
    """v5 kernel; see module docstring. Probe modes:
      full    - everything
      dmaonly - x DMAs only
      noxdma  - MMs+reducers from undisturbed SBUF (no x DMA)
      nored   - DMAs + MMs, no reducers
      pedma   - DMAs + MMs + no reducers (alias nored)
    """
    from concourse import bacc, mybir, tile

    f32 = mybir.dt.float32
    bf16 = mybir.dt.bfloat16
    Act = mybir.ActivationFunctionType
    Alu = mybir.AluOpType

    assert sum(sched) == _BS
    n_groups = _NCH // gsz
    n_act_of, act_order, dve_order = _assign(act_chunks, gsz)
    n_act_total = len(act_order)
    n_dve_total = len(dve_order)

    nc = bacc.Bacc("TRN2", target_bir_lowering=False, debug=False)

    # x^T and Q' with the 256-row contraction dim split into 2 blocks of
    # 128 partitions; [blk, 128, cols] in DRAM, loaded as [128, blk, cols]
    # so each slab (both blocks) is a single DMA instruction.
    xt = nc.dram_tensor("xt", [2, 128, _BS], bf16, kind="ExternalInput").ap()
    qp = nc.dram_tensor("qp", [2, 128, _N], bf16, kind="ExternalInput").ap()
    # y[p, k] = sum_n z^2 for engine-ordered chunk k (ACT cols, then DVE)
    y = nc.dram_tensor("y", [128, _NCH], f32, kind="ExternalOutput").ap()

    max_dch = max(sched)
    dma_engines = [nc.sync, nc.scalar] if dma_alt else [nc.sync]

    with tile.TileContext(nc) as tc:
        with (
            tc.tile_pool(name="const", bufs=1) as cpool,
            tc.tile_pool(name="xin", bufs=xin_bufs) as xpool,
            tc.tile_pool(name="yout", bufs=2) as ypool,
            tc.tile_pool(name="stat", bufs=2) as stpool,
            tc.tile_pool(name="scr", bufs=4) as spool,
            tc.tile_pool(name="fix", bufs=2) as fpool,
            tc.tile_pool(name="pz", bufs=pz_bufs, space="PSUM") as pzpool,
        ):
            q_sb = cpool.tile([128, 2, _N], bf16)
            nc.sync.dma_start(q_sb[:], qp[:, :, :].transpose([1, 0, 2]))

            def emit_group(g, pz, y_sb, st_sb, cursors):
                n_act = n_act_of[g]
                for j in range(n_act):
                    scr = spool.tile([128, _N], bf16, tag="s")
                    nc.scalar.activation(
                        scr[:], pz[:, j, :], Act.Square,
                        accum_out=y_sb[:, cursors[0]:cursors[0] + 1])
                    cursors[0] += 1
                for j in range(n_act, gsz):
                    d = cursors[1]
                    nc.vector.bn_stats(st_sb[:, d, :], pz[:, j, :])
                    cursors[1] += 1

            for _rep in range(reps):
                y_sb = ypool.tile([128, _NCH], f32, tag="y")
                st_sb = stpool.tile([128, max(n_dve_total, 1), 6], f32,
                                    tag="st")
                cursors = [0, 0]   # [next ACT y col, next stats slot]
                c = 0
                off = 0
                dma_i = 0
                for dch in sched:
                    x_sb = xpool.tile([128, 2, max_dch], bf16, tag="x")
                    if mode != "noxdma":
                        eng = dma_engines[dma_i % len(dma_engines)]
                        eng.dma_start(
                            x_sb[:, :, 0:dch],
                            xt[:, :, off:off + dch].transpose([1, 0, 2]))
                        dma_i += 1
                    off += dch
                    if mode == "dmaonly":
                        continue
                    for k in range(dch // 128):
                        j = c % gsz
                        if j == 0:
                            pz = pzpool.tile([128, gsz, _N], f32, tag="pz")
                        nc.tensor.matmul(pz[:, j, :],
                                         x_sb[:, 0, k * 128:(k + 1) * 128],
                                         q_sb[:, 0, :], start=True, stop=False)
                        nc.tensor.matmul(pz[:, j, :],
                                         x_sb[:, 1, k * 128:(k + 1) * 128],
                                         q_sb[:, 1, :], start=False, stop=True)
                        c += 1
                        if mode in ("nored", "pedma"):
                            continue
                        if j == gsz - 1:
                            emit_group(c // gsz - 1, pz, y_sb, st_sb, cursors)
                if mode in ("full",):
                    # fixup: y_dve = cv_e + cv_o + 128*(me^2 + mo^2)
                    nd = n_dve_total
                    if nd:
                        u = fpool.tile([128, nd, 4], f32, tag="u")
                        # u0 = cv_e + cv_o ; u1 = me*me ; u2 = mo*mo
                        nc.vector.tensor_tensor(
                            out=u[:, :, 0], in0=st_sb[:, :, 2],
                            in1=st_sb[:, :, 5], op=Alu.add)
                        nc.vector.tensor_tensor(
                            out=u[:, :, 1], in0=st_sb[:, :, 1],
                            in1=st_sb[:, :, 1], op=Alu.mult)
                        nc.vector.tensor_tensor(
                            out=u[:, :, 2], in0=st_sb[:, :, 4],
                            in1=st_sb[:, :, 4], op=Alu.mult)
                        nc.vector.tensor_tensor(
                            out=u[:, :, 3], in0=u[:, :, 1],
                            in1=u[:, :, 2], op=Alu.add)
                        nc.vector.scalar_tensor_tensor(
                            out=y_sb[:, n_act_total:n_act_total + nd],
                            in0=u[:, :, 3], scalar=128.0, in1=u[:, :, 0],
                            op0=Alu.mult, op1=Alu.add)
                    nc.sync.dma_start(y[:, :], y_sb[:])

    nc.compile()
    return nc


def _get_nc(reps=1, **kw):
    key = (reps,) + tuple(sorted(kw.items()))
    if key not in _compiled_nc:
        _compiled_nc[key] = _build_nc(reps, **kw)
    return _compiled_nc[key]


def _to_bf16(a):
    import ml_dtypes

    return np.ascontiguousarray(a).astype(ml_dtypes.bfloat16)


def _host_prep(x, w_lin, b_lin, v, field_idx):
    """Host-side tiny-param preprocessing + sharding. Returns (in_maps, lin)."""
    x = np.asarray(x, dtype=np.float32)
    w_lin = np.asarray(w_lin, dtype=np.float64)
    b_lin = np.asarray(b_lin, dtype=np.float64)
    v = np.asarray(v, dtype=np.float64)
    field_idx = np.asarray(field_idx, dtype=np.int64)

    # Wmat[i, j] = <v[i, field[j]], v[j, field[i]]>
    A = v[:, field_idx, :]                       # [N, N, K]
    Wmat = np.einsum('ijk,jik->ij', A, A)        # [N, N]
    Wu = np.triu(Wmat, 1)
    S = (Wu + Wu.T) * 0.5
    lam, Q = np.linalg.eigh(S)                   # ascending eigenvalues
    c = max(0.0, -lam[0])
    mu = np.clip(lam + c, 0.0, None)
    # [N, N] column-scaled, contraction split into 2 blocks of 128 rows
    Qp = _to_bf16(Q * np.sqrt(mu)[None, :]).reshape(2, 128, _N)

    # x transposed + sharded along batch, bf16, [2, 128, BS] per core
    x64 = x.astype(np.float64)
    xts = x.reshape(_NCORES, _BS, _N).transpose(0, 2, 1)  # [8, N, BS]
    xts = _to_bf16(xts).reshape(_NCORES, 2, 128, _BS)

    in_maps = [{"xt": xts[i], "qp": Qp} for i in range(_NCORES)]
    # linear part and the -c||x||^2 spectrum-shift correction, both host-side
    lin = x64 @ w_lin + b_lin[0] - c * np.einsum('bi,bi->b', x64, x64)
    return in_maps, lin


def _y_perm(act_chunks=28, gsz=4):
    """Column permutation: y_sb engine-order col -> chunk id."""
    _, act_order, dve_order = _assign(act_chunks, gsz)
    return np.array(act_order + dve_order)


def _run_device(in_maps, trace=False, reps=1, **kw):
    from concourse.bass_utils import run_bass_kernel_spmd

    nc = _get_nc(reps, **kw)
    res = run_bass_kernel_spmd(
        nc, in_maps, core_ids=list(range(_NCORES)), trace=trace
    )
    perm = _y_perm(kw.get("act_chunks", 28), kw.get("gsz", 4))
    inv = np.empty_like(perm)
    inv[perm] = np.arange(len(perm))
    # y[p, k] (engine order) -> chunk order -> batch order
    yq = np.concatenate(
        [np.asarray(res.results[i]["y"], dtype=np.float64)[:, inv]
         .T.reshape(-1)
         for i in range(_NCORES)]
    )
    return yq, res


def kernel(x, w_lin, b_lin, v, field_idx):
    in_maps, lin = _host_prep(x, w_lin, b_lin, v, field_idx)
    yq, _ = _run_device(in_maps, trace=False)
    return (lin + yq).astype(np.float32)[:, None]


# revision 13
# speedup vs baseline: 1.1859x; 1.1859x over previous
"""FFM (field-aware factorization machine) forward kernel for 8 TRN2 NeuronCores.

y[b] = x[b] @ w_lin + b_lin + sum_{i<j} Wu[i,j] x[b,i] x[b,j]
with Wu = triu(Wmat, 1), Wmat[i,j] = <v[i, field[j]], v[j, field[i]]>.

Strategy (v5):
  - Host: build Wmat from (v, field_idx) [tiny], symmetrize
    S = (Wu + Wu^T)/2, eigendecompose S = Q diag(lam) Q^T. Shift the
    spectrum by c = -lam_min so mu = lam + c >= 0, fold sqrt(mu) into the
    eigenvectors: Q' = Q diag(sqrt(mu)). Then
      x^T Wu x = sum_n mu_n (x . q_n)^2 - c ||x||^2
    and the -c||x||^2 correction joins the (host-computed) linear part.
  - Device (data-parallel over batch, 8 cores): per 128-sample chunk,
    PE computes z = x_chunk^T Q' with batch on PSUM partitions and the
    eigen index on the free dim (two bf16 matmuls, contraction 256).
    The per-sample reduction sum_n z_n^2 runs along the FREE dim, with
    whole PSUM GROUPS assigned to one engine (group-level split avoids
    ACT/DVE hammering the same PSUM banks):
      * ACT groups: ScalarE Square(psum)+accum_out per chunk.
      * DVE groups: VectorE bn_stats per chunk straight from PSUM;
        sum z^2 is reconstructed from the even/odd (count,mean,
        count*var) stats in one batched 5-op fixup at rep end:
          sum z^2 = cv_e + 128 me^2 + cv_o + 128 mo^2.
        This avoids the PSUM->SBUF copy entirely (PSUM may be read
        once per DVE instr; bn_stats needs it only once).
  - x ships as bf16 pre-transposed with each DMA slab fully contiguous
    per partition row ([128, 2*dch] 4KB descriptors).
  - y columns are engine-ordered (ACT cols first, then DVE cols);
    the host inverts the permutation.
"""

import numpy as np

_LDW_OPT = {"on": False}


def _install_walrus_patch():
    """Allow flipping walrus --enable-ldw-opt at NEFF-compile time."""
    from concourse import bass_utils
    if getattr(bass_utils, "_ant_ldw_patched", False):
        return
    orig = bass_utils.run_command

    def patched(cmd, *a, **kw):
        if _LDW_OPT["on"] and isinstance(cmd, list):
            cmd = [c.replace("--enable-ldw-opt=false", "--enable-ldw-opt=true")
                   if isinstance(c, str) else c for c in cmd]
        return orig(cmd, *a, **kw)

    bass_utils.run_command = patched
    bass_utils._ant_ldw_patched = True


_B, _N = 65536, 256
_NCORES = 8
_BS = _B // _NCORES   # 8192 samples per core
_NCH = _BS // 128     # 64 batch chunks per core
_DCH = 1024           # DMA slab columns
_NSLAB = _BS // _DCH

_compiled_nc = {}


def _assign(act_chunks, gsz, nch=_NCH):
    """Group-level engine assignment: whole groups go to ACT until
    act_chunks is covered (rounded to groups), interleaved evenly.

    Returns (n_act_of[g], act_order, dve_order)."""
    n_groups = nch // gsz
    n_act_groups = min(n_groups, round(act_chunks / gsz))
    # spread ACT groups evenly among all groups
    is_act = [False] * n_groups
    if n_act_groups:
        for i in range(n_act_groups):
            is_act[(i * n_groups) // n_act_groups] = True
    n_act_of = [gsz if a else 0 for a in is_act]
    act_order, dve_order = [], []
    for g in range(n_groups):
        for j in range(gsz):
            c = g * gsz + j
            (act_order if j < n_act_of[g] else dve_order).append(c)
    return n_act_of, act_order, dve_order


def _build_nc(reps=1, mode="full", act_chunks=24, xin_bufs=3,
              gsz=2, pz_bufs=8, dch=2048, dma_alt=True, xlayout="slab",
              ldw_opt=0, salt=0):
    """v5 kernel; see module docstring. Probe modes:
      full    - everything
      dmaonly - x DMAs only
      noxdma  - MMs+reducers from undisturbed SBUF (no x DMA)
      nored   - DMAs + MMs, no reducers
      samew   - DMAs + MMs with a CONSTANT stationary operand (junk
                math; probes whether unchanged weights skip/hide LDW)
      samewnx - samew without x DMAs (pure-PE constant-weight probe)
      mmonly  - real MM structure without x DMAs or reducers
    """
    from concourse import bacc, mybir, tile

    f32 = mybir.dt.float32
    bf16 = mybir.dt.bfloat16
    Act = mybir.ActivationFunctionType
    Alu = mybir.AluOpType

    assert _BS % dch == 0
    nslab = _BS // dch
    n_groups = _NCH // gsz
    n_act_of, act_order, dve_order = _assign(act_chunks, gsz)
    n_act_total = len(act_order)
    n_dve_total = len(dve_order)

    _install_walrus_patch()
    _LDW_OPT["on"] = bool(ldw_opt)

    nc = bacc.Bacc("TRN2", target_bir_lowering=False, debug=False)

    # salt defeats the NEFF cache when only compile flags change
    if salt:
        nc.dram_tensor(f"salt{salt}", [1, 1], mybir.dt.float32,
                       kind="ExternalOutput")

    # x pre-transposed. Two DRAM layouts:
    #   row:  [2, 128, _BS] row-major; a slab reads a column range of
    #         every (blk, p) row -> 2*128 descriptors of dch*2B, 16KB
    #         DRAM stride between descriptors.
    #   slab: [nslab, 128, 2, dch]; slab s partition p is contiguous
    #         (2*dch*2B descriptors).
    if xlayout == "row":
        xt = nc.dram_tensor("xt", [2, 128, _BS], bf16,
                            kind="ExternalInput").ap()
    else:
        xt = nc.dram_tensor("xt", [nslab, 128, 2, dch], bf16,
                            kind="ExternalInput").ap()

    def x_slab_src(s):
        if xlayout == "row":
            return xt[:, :, s * dch:(s + 1) * dch].transpose([1, 0, 2])
        return xt[s]
    qp = nc.dram_tensor("qp", [2, 128, _N], bf16, kind="ExternalInput").ap()
    # y[p, k] = sum_n z^2 for engine-ordered chunk k (ACT cols, then DVE)
    y = nc.dram_tensor("y", [128, _NCH], f32, kind="ExternalOutput").ap()

    dma_engines = [nc.sync, nc.scalar] if dma_alt else [nc.sync]

    with tile.TileContext(nc) as tc:
        with (
            tc.tile_pool(name="const", bufs=1) as cpool,
            tc.tile_pool(name="xin", bufs=xin_bufs) as xpool,
            tc.tile_pool(name="yout", bufs=2) as ypool,
            tc.tile_pool(name="stat", bufs=2) as stpool,
            tc.tile_pool(name="scr", bufs=4) as spool,
            tc.tile_pool(name="fix", bufs=2) as fpool,
            tc.tile_pool(name="pz", bufs=pz_bufs, space="PSUM") as pzpool,
        ):
            q_sb = cpool.tile([128, 2, _N], bf16)
            nc.sync.dma_start(q_sb[:], qp[:, :, :].transpose([1, 0, 2]))
            if mode in ("noxdma", "samewnx", "mmonly"):
                xfix = cpool.tile([128, 2, dch], bf16)
                nc.sync.dma_start(xfix[:], x_slab_src(0))

            def emit_group(g, pz, y_sb, st_sb, cursors):
                n_act = n_act_of[g]
                for j in range(n_act):
                    scr = spool.tile([128, _N], bf16, tag="s")
                    nc.scalar.activation(
                        scr[:], pz[:, j, :], Act.Square,
                        accum_out=y_sb[:, cursors[0]:cursors[0] + 1])
                    cursors[0] += 1
                for j in range(n_act, gsz):
                    nc.vector.bn_stats(st_sb[:, cursors[1], :], pz[:, j, :])
                    cursors[1] += 1

            for _rep in range(reps):
                y_sb = ypool.tile([128, _NCH], f32, tag="y")
                st_sb = stpool.tile([128, max(n_dve_total, 1), 6], f32,
                                    tag="st")
                cursors = [0, 0]   # [next ACT y col, next stats slot]
                c = 0
                for s in range(nslab):
                    if mode in ("noxdma", "samewnx", "mmonly"):
                        x_sb = xfix
                    else:
                        x_sb = xpool.tile([128, 2, dch], bf16, tag="x")
                        eng = dma_engines[s % len(dma_engines)]
                        eng.dma_start(x_sb[:], x_slab_src(s))
                    for k in range(dch // 128):
                        j = c % gsz
                        if j == 0:
                            pz = pzpool.tile([128, gsz, _N], f32, tag="pz")
                        if mode in ("samew", "samewnx"):
                            nc.tensor.matmul(pz[:, j, :],
                                             q_sb[:, 0, 0:128],
                                             x_sb[:, 0, :].rearrange(
                                                 "p (a b) -> p a b", b=256)
                                             [:, k % (dch // 256), :],
                                             start=True, stop=True)
                            c += 1
                            continue
                        nc.tensor.matmul(pz[:, j, :],
                                         x_sb[:, 0, k * 128:(k + 1) * 128],
                                         q_sb[:, 0, :], start=True, stop=False)
                        nc.tensor.matmul(pz[:, j, :],
                                         x_sb[:, 1, k * 128:(k + 1) * 128],
                                         q_sb[:, 1, :], start=False, stop=True)
                        c += 1
                        if mode in ("nored", "samew", "samewnx", "mmonly"):
                            continue
                        if j == gsz - 1:
                            emit_group(c // gsz - 1, pz, y_sb, st_sb, cursors)
                if mode == "full":
                    # fixup: y_dve = cv_e + cv_o + 128*(me^2 + mo^2)
                    nd = n_dve_total
                    if nd:
                        u = fpool.tile([128, nd, 4], f32, tag="u")
                        nc.vector.tensor_tensor(
                            out=u[:, :, 0], in0=st_sb[:, :, 2],
                            in1=st_sb[:, :, 5], op=Alu.add)
                        nc.vector.tensor_tensor(
                            out=u[:, :, 1], in0=st_sb[:, :, 1],
                            in1=st_sb[:, :, 1], op=Alu.mult)
                        nc.vector.tensor_tensor(
                            out=u[:, :, 2], in0=st_sb[:, :, 4],
                            in1=st_sb[:, :, 4], op=Alu.mult)
                        nc.vector.tensor_tensor(
                            out=u[:, :, 3], in0=u[:, :, 1],
                            in1=u[:, :, 2], op=Alu.add)
                        nc.vector.scalar_tensor_tensor(
                            out=y_sb[:, n_act_total:n_act_total + nd],
                            in0=u[:, :, 3], scalar=128.0, in1=u[:, :, 0],
                            op0=Alu.mult, op1=Alu.add)
                    nc.sync.dma_start(y[:, :], y_sb[:])

    nc.compile()
    return nc


def _get_nc(reps=1, **kw):
    key = (reps,) + tuple(sorted(kw.items()))
    if key not in _compiled_nc:
        _compiled_nc[key] = _build_nc(reps, **kw)
    return _compiled_nc[key]


def _to_bf16(a):
    import ml_dtypes

    return np.ascontiguousarray(a).astype(ml_dtypes.bfloat16)


def _host_prep(x, w_lin, b_lin, v, field_idx, dch=2048, xlayout="slab"):
    """Host-side tiny-param preprocessing + sharding. Returns (in_maps, lin)."""
    x = np.asarray(x, dtype=np.float32)
    w_lin = np.asarray(w_lin, dtype=np.float64)
    b_lin = np.asarray(b_lin, dtype=np.float64)
    v = np.asarray(v, dtype=np.float64)
    field_idx = np.asarray(field_idx, dtype=np.int64)

    # Wmat[i, j] = <v[i, field[j]], v[j, field[i]]>
    A = v[:, field_idx, :]                       # [N, N, K]
    Wmat = np.einsum('ijk,jik->ij', A, A)        # [N, N]
    Wu = np.triu(Wmat, 1)
    S = (Wu + Wu.T) * 0.5
    lam, Q = np.linalg.eigh(S)                   # ascending eigenvalues
    c = max(0.0, -lam[0])
    mu = np.clip(lam + c, 0.0, None)
    # [N, N] column-scaled, contraction split into 2 blocks of 128 rows
    Qp = _to_bf16(Q * np.sqrt(mu)[None, :]).reshape(2, 128, _N)

    nslab = _BS // dch
    x64 = x.astype(np.float64)
    if xlayout == "row":
        # [core, blk, p, col] = x^T split into 2 contraction blocks
        xts = x.reshape(_NCORES, _BS, _N).transpose(0, 2, 1)  # [8, N, BS]
        xts = _to_bf16(xts).reshape(_NCORES, 2, 128, _BS)
    else:
        # [core, slab, p, blk, col]: (blk, col) contiguous per partition
        xts = _to_bf16(x).reshape(_NCORES, nslab, dch, 2, 128)
        xts = np.ascontiguousarray(xts.transpose(0, 1, 4, 3, 2))

    in_maps = [{"xt": xts[i], "qp": Qp} for i in range(_NCORES)]
    # linear part and the -c||x||^2 spectrum-shift correction, both host-side
    lin = x64 @ w_lin + b_lin[0] - c * np.einsum('bi,bi->b', x64, x64)
    return in_maps, lin


def _y_perm(act_chunks=24, gsz=2):
    """Column permutation: y_sb engine-order col -> chunk id."""
    _, act_order, dve_order = _assign(act_chunks, gsz)
    return np.array(act_order + dve_order)


def _run_device(in_maps, trace=False, reps=1, **kw):
    from concourse.bass_utils import run_bass_kernel_spmd

    nc = _get_nc(reps, **kw)
    res = run_bass_kernel_spmd(
        nc, in_maps, core_ids=list(range(_NCORES)), trace=trace
    )
    perm = _y_perm(kw.get("act_chunks", 24), kw.get("gsz", 2))
    inv = np.empty_like(perm)
    inv[perm] = np.arange(len(perm))
    # y[p, k] (engine order) -> chunk order -> batch order
    yq = np.concatenate(
        [np.asarray(res.results[i]["y"], dtype=np.float64)[:, inv]
         .T.reshape(-1)
         for i in range(_NCORES)]
    )
    return yq, res


def kernel(x, w_lin, b_lin, v, field_idx):
    in_maps, lin = _host_prep(x, w_lin, b_lin, v, field_idx)
    yq, _ = _run_device(in_maps, trace=False)
    return (lin + yq).astype(np.float32)[:, None]


# revision 14
# speedup vs baseline: 1.5379x; 1.2968x over previous
"""FFM (field-aware factorization machine) forward kernel for 8 TRN2 NeuronCores.

y[b] = x[b] @ w_lin + b_lin + sum_{i<j} Wu[i,j] x[b,i] x[b,j]
with Wu = triu(Wmat, 1), Wmat[i,j] = <v[i, field[j]], v[j, field[i]]>.

Strategy (v5):
  - Host: build Wmat from (v, field_idx) [tiny], symmetrize
    S = (Wu + Wu^T)/2, eigendecompose S = Q diag(lam) Q^T. Shift the
    spectrum by c = -lam_min so mu = lam + c >= 0, fold sqrt(mu) into the
    eigenvectors: Q' = Q diag(sqrt(mu)). Then
      x^T Wu x = sum_n mu_n (x . q_n)^2 - c ||x||^2
    and the -c||x||^2 correction joins the (host-computed) linear part.
  - Device (data-parallel over batch, 8 cores): per 128-sample chunk,
    PE computes z = x_chunk^T Q' with batch on PSUM partitions and the
    eigen index on the free dim (two bf16 matmuls, contraction 256).
    The per-sample reduction sum_n z_n^2 runs along the FREE dim, with
    whole PSUM GROUPS assigned to one engine (group-level split avoids
    ACT/DVE hammering the same PSUM banks):
      * ACT groups: ScalarE Square(psum)+accum_out per chunk.
      * DVE groups: VectorE bn_stats per chunk straight from PSUM;
        sum z^2 is reconstructed from the even/odd (count,mean,
        count*var) stats in one batched 5-op fixup at rep end:
          sum z^2 = cv_e + 128 me^2 + cv_o + 128 mo^2.
        This avoids the PSUM->SBUF copy entirely (PSUM may be read
        once per DVE instr; bn_stats needs it only once).
  - x ships as bf16 pre-transposed with each DMA slab fully contiguous
    per partition row ([128, 2*dch] 4KB descriptors).
  - y columns are engine-ordered (ACT cols first, then DVE cols);
    the host inverts the permutation.
"""

import numpy as np

_LDW_OPT = {"on": False}


def _install_walrus_patch():
    """Allow flipping walrus --enable-ldw-opt at NEFF-compile time."""
    from concourse import bass_utils
    if getattr(bass_utils, "_ant_ldw_patched", False):
        return
    orig = bass_utils.run_command

    def patched(cmd, *a, **kw):
        if _LDW_OPT["on"] and isinstance(cmd, list):
            cmd = [c.replace("--enable-ldw-opt=false", "--enable-ldw-opt=true")
                   if isinstance(c, str) else c for c in cmd]
        return orig(cmd, *a, **kw)

    bass_utils.run_command = patched
    bass_utils._ant_ldw_patched = True


_B, _N = 65536, 256
_NCORES = 8
_BS = _B // _NCORES   # 8192 samples per core
_NCH = _BS // 128     # 64 batch chunks per core
_DCH = 1024           # DMA slab columns
_NSLAB = _BS // _DCH

_compiled_nc = {}


def _assign(act_chunks, gsz, nch=_NCH):
    """Group-level engine assignment: whole groups go to ACT until
    act_chunks is covered (rounded to groups), interleaved evenly.

    Returns (n_act_of[g], act_order, dve_order)."""
    n_groups = nch // gsz
    n_act_groups = min(n_groups, round(act_chunks / gsz))
    # spread ACT groups evenly among all groups
    is_act = [False] * n_groups
    if n_act_groups:
        for i in range(n_act_groups):
            is_act[(i * n_groups) // n_act_groups] = True
    n_act_of = [gsz if a else 0 for a in is_act]
    act_order, dve_order = [], []
    for g in range(n_groups):
        for j in range(gsz):
            c = g * gsz + j
            (act_order if j < n_act_of[g] else dve_order).append(c)
    return n_act_of, act_order, dve_order


def _build_nc(reps=1, mode="full", act_chunks=24, xin_bufs=8,
              gsz=2, pz_bufs=8, dch=1024, dma_alt=False, xlayout="slab",
              ldw_opt=0, salt=0):
    """v5 kernel; see module docstring. Probe modes:
      full    - everything
      dmaonly - x DMAs only
      noxdma  - MMs+reducers from undisturbed SBUF (no x DMA)
      nored   - DMAs + MMs, no reducers
      samew   - DMAs + MMs with a CONSTANT stationary operand (junk
                math; probes whether unchanged weights skip/hide LDW)
      samewnx - samew without x DMAs (pure-PE constant-weight probe)
      mmonly  - real MM structure without x DMAs or reducers
    """
    from concourse import bacc, mybir, tile

    f32 = mybir.dt.float32
    bf16 = mybir.dt.bfloat16
    Act = mybir.ActivationFunctionType
    Alu = mybir.AluOpType

    assert _BS % dch == 0
    nslab = _BS // dch
    n_groups = _NCH // gsz
    n_act_of, act_order, dve_order = _assign(act_chunks, gsz)
    n_act_total = len(act_order)
    n_dve_total = len(dve_order)

    _install_walrus_patch()
    _LDW_OPT["on"] = bool(ldw_opt)

    nc = bacc.Bacc("TRN2", target_bir_lowering=False, debug=False)

    # salt defeats the NEFF cache when only compile flags change
    if salt:
        nc.dram_tensor(f"salt{salt}", [1, 1], mybir.dt.float32,
                       kind="ExternalOutput")

    # x pre-transposed. Two DRAM layouts:
    #   row:  [2, 128, _BS] row-major; a slab reads a column range of
    #         every (blk, p) row -> 2*128 descriptors of dch*2B, 16KB
    #         DRAM stride between descriptors.
    #   slab: [nslab, 128, 2, dch]; slab s partition p is contiguous
    #         (2*dch*2B descriptors).
    if xlayout == "row":
        xt = nc.dram_tensor("xt", [2, 128, _BS], bf16,
                            kind="ExternalInput").ap()
    else:
        xt = nc.dram_tensor("xt", [nslab, 128, 2, dch], bf16,
                            kind="ExternalInput").ap()

    def x_slab_src(s):
        if xlayout == "row":
            return xt[:, :, s * dch:(s + 1) * dch].transpose([1, 0, 2])
        return xt[s]
    qp = nc.dram_tensor("qp", [2, 128, _N], bf16, kind="ExternalInput").ap()
    # y[p, k] = sum_n z^2 for engine-ordered chunk k (ACT cols, then DVE)
    y = nc.dram_tensor("y", [128, _NCH], f32, kind="ExternalOutput").ap()

    dma_engines = [nc.sync, nc.scalar] if dma_alt else [nc.sync]

    with tile.TileContext(nc) as tc:
        with (
            tc.tile_pool(name="const", bufs=1) as cpool,
            tc.tile_pool(name="xin", bufs=xin_bufs) as xpool,
            tc.tile_pool(name="yout", bufs=2) as ypool,
            tc.tile_pool(name="stat", bufs=2) as stpool,
            tc.tile_pool(name="scr", bufs=4) as spool,
            tc.tile_pool(name="fix", bufs=2) as fpool,
            tc.tile_pool(name="pz", bufs=pz_bufs, space="PSUM") as pzpool,
        ):
            q_sb = cpool.tile([128, 2, _N], bf16)
            nc.sync.dma_start(q_sb[:], qp[:, :, :].transpose([1, 0, 2]))
            if mode in ("noxdma", "samewnx", "mmonly"):
                xfix = cpool.tile([128, 2, dch], bf16)
                nc.sync.dma_start(xfix[:], x_slab_src(0))

            def emit_group(g, pz, y_sb, st_sb, cursors):
                n_act = n_act_of[g]
                for j in range(n_act):
                    scr = spool.tile([128, _N], bf16, tag="s")
                    nc.scalar.activation(
                        scr[:], pz[:, j, :], Act.Square,
                        accum_out=y_sb[:, cursors[0]:cursors[0] + 1])
                    cursors[0] += 1
                for j in range(n_act, gsz):
                    nc.vector.bn_stats(st_sb[:, cursors[1], :], pz[:, j, :])
                    cursors[1] += 1

            for _rep in range(reps):
                y_sb = ypool.tile([128, _NCH], f32, tag="y")
                st_sb = stpool.tile([128, max(n_dve_total, 1), 6], f32,
                                    tag="st")
                cursors = [0, 0]   # [next ACT y col, next stats slot]
                c = 0
                for s in range(nslab):
                    if mode in ("noxdma", "samewnx", "mmonly"):
                        x_sb = xfix
                    else:
                        x_sb = xpool.tile([128, 2, dch], bf16, tag="x")
                        eng = dma_engines[s % len(dma_engines)]
                        eng.dma_start(x_sb[:], x_slab_src(s))
                    for k in range(dch // 128):
                        j = c % gsz
                        if j == 0:
                            pz = pzpool.tile([128, gsz, _N], f32, tag="pz")
                        if mode in ("samew", "samewnx"):
                            nc.tensor.matmul(pz[:, j, :],
                                             q_sb[:, 0, 0:128],
                                             x_sb[:, 0, :].rearrange(
                                                 "p (a b) -> p a b", b=256)
                                             [:, k % (dch // 256), :],
                                             start=True, stop=True)
                            c += 1
                            continue
                        nc.tensor.matmul(pz[:, j, :],
                                         x_sb[:, 0, k * 128:(k + 1) * 128],
                                         q_sb[:, 0, :], start=True, stop=False)
                        nc.tensor.matmul(pz[:, j, :],
                                         x_sb[:, 1, k * 128:(k + 1) * 128],
                                         q_sb[:, 1, :], start=False, stop=True)
                        c += 1
                        if mode in ("nored", "samew", "samewnx", "mmonly"):
                            continue
                        if j == gsz - 1:
                            emit_group(c // gsz - 1, pz, y_sb, st_sb, cursors)
                if mode == "full":
                    # fixup: y_dve = cv_e + cv_o + 128*(me^2 + mo^2)
                    nd = n_dve_total
                    if nd:
                        u = fpool.tile([128, nd, 4], f32, tag="u")
                        nc.vector.tensor_tensor(
                            out=u[:, :, 0], in0=st_sb[:, :, 2],
                            in1=st_sb[:, :, 5], op=Alu.add)
                        nc.vector.tensor_tensor(
                            out=u[:, :, 1], in0=st_sb[:, :, 1],
                            in1=st_sb[:, :, 1], op=Alu.mult)
                        nc.vector.tensor_tensor(
                            out=u[:, :, 2], in0=st_sb[:, :, 4],
                            in1=st_sb[:, :, 4], op=Alu.mult)
                        nc.vector.tensor_tensor(
                            out=u[:, :, 3], in0=u[:, :, 1],
                            in1=u[:, :, 2], op=Alu.add)
                        nc.vector.scalar_tensor_tensor(
                            out=y_sb[:, n_act_total:n_act_total + nd],
                            in0=u[:, :, 3], scalar=128.0, in1=u[:, :, 0],
                            op0=Alu.mult, op1=Alu.add)
                    nc.sync.dma_start(y[:, :], y_sb[:])

    nc.compile()
    return nc


def _get_nc(reps=1, **kw):
    key = (reps,) + tuple(sorted(kw.items()))
    if key not in _compiled_nc:
        _compiled_nc[key] = _build_nc(reps, **kw)
    return _compiled_nc[key]


def _to_bf16(a):
    import ml_dtypes

    return np.ascontiguousarray(a).astype(ml_dtypes.bfloat16)


def _host_prep(x, w_lin, b_lin, v, field_idx, dch=1024, xlayout="slab"):
    """Host-side tiny-param preprocessing + sharding. Returns (in_maps, lin)."""
    x = np.asarray(x, dtype=np.float32)
    w_lin = np.asarray(w_lin, dtype=np.float64)
    b_lin = np.asarray(b_lin, dtype=np.float64)
    v = np.asarray(v, dtype=np.float64)
    field_idx = np.asarray(field_idx, dtype=np.int64)

    # Wmat[i, j] = <v[i, field[j]], v[j, field[i]]>
    A = v[:, field_idx, :]                       # [N, N, K]
    Wmat = np.einsum('ijk,jik->ij', A, A)        # [N, N]
    Wu = np.triu(Wmat, 1)
    S = (Wu + Wu.T) * 0.5
    lam, Q = np.linalg.eigh(S)                   # ascending eigenvalues
    c = max(0.0, -lam[0])
    mu = np.clip(lam + c, 0.0, None)
    # [N, N] column-scaled, contraction split into 2 blocks of 128 rows
    Qp = _to_bf16(Q * np.sqrt(mu)[None, :]).reshape(2, 128, _N)

    nslab = _BS // dch
    x64 = x.astype(np.float64)
    if xlayout == "row":
        # [core, blk, p, col] = x^T split into 2 contraction blocks
        xts = x.reshape(_NCORES, _BS, _N).transpose(0, 2, 1)  # [8, N, BS]
        xts = _to_bf16(xts).reshape(_NCORES, 2, 128, _BS)
    else:
        # [core, slab, p, blk, col]: (blk, col) contiguous per partition
        xts = _to_bf16(x).reshape(_NCORES, nslab, dch, 2, 128)
        xts = np.ascontiguousarray(xts.transpose(0, 1, 4, 3, 2))

    in_maps = [{"xt": xts[i], "qp": Qp} for i in range(_NCORES)]
    # linear part and the -c||x||^2 spectrum-shift correction, both host-side
    lin = x64 @ w_lin + b_lin[0] - c * np.einsum('bi,bi->b', x64, x64)
    return in_maps, lin


def _y_perm(act_chunks=24, gsz=2):
    """Column permutation: y_sb engine-order col -> chunk id."""
    _, act_order, dve_order = _assign(act_chunks, gsz)
    return np.array(act_order + dve_order)


def _run_device(in_maps, trace=False, reps=1, **kw):
    from concourse.bass_utils import run_bass_kernel_spmd

    nc = _get_nc(reps, **kw)
    res = run_bass_kernel_spmd(
        nc, in_maps, core_ids=list(range(_NCORES)), trace=trace
    )
    perm = _y_perm(kw.get("act_chunks", 24), kw.get("gsz", 2))
    inv = np.empty_like(perm)
    inv[perm] = np.arange(len(perm))
    # y[p, k] (engine order) -> chunk order -> batch order
    yq = np.concatenate(
        [np.asarray(res.results[i]["y"], dtype=np.float64)[:, inv]
         .T.reshape(-1)
         for i in range(_NCORES)]
    )
    return yq, res


def kernel(x, w_lin, b_lin, v, field_idx):
    in_maps, lin = _host_prep(x, w_lin, b_lin, v, field_idx)
    yq, _ = _run_device(in_maps, trace=False)
    return (lin + yq).astype(np.float32)[:, None]
